# revision 54
# baseline (speedup 1.0000x reference)
"""Causal self-attention (GQA + RoPE) Trainium2 Bass kernel.

Problem: B=2, T=2048, C=2048, H=16 q-heads, HK=4 kv-heads, HD=128.
Sharding: 8 cores = (batch b in {0,1}) x (kv-head group g in {0..3}).
Each core computes its batch's 4 q-heads / 1 kv-head slice end-to-end
(QKV proj -> RoPE -> causal attention -> o-proj partial), returning a
[T, C] partial y; the host sums the 4 group partials per batch.

Schedule (~488k PE cycles vs 537k in the v1 baseline; TimelineSim
240.6us vs 275.4us):
 - Softmax denominators: ex blocks are tree-summed in bf16 on DVE per
   group of <=4 tk blocks, then ONE ones-matmul per group feeds the
   PSUM row accumulator -- 40 instead of 160 ones-matmuls (~20us PE).
 - Causal masking: post-exp Pool affine_select on the diagonal 128x128
   sub-block (+ Pool memset of the dead prefix) instead of a -1e30
   DVE add on PSUM; frees ~40us of DVE.
 - HARD HW CONSTRAINT (not modeled by the simulator): only one matmul
   accumulation chain may be open per PSUM bank at a time; interleaved
   chains in one bank silently corrupt all but the last.  So wave-1
   rides the xT stream kc-major with K(j0,j1) + Q(h0,j0) + V blocks
   0-3 in SOLO banks, and V4-15 run as 12 solo kc-inner chains at the
   gate, rotating through banks exactly as the bias-adds/ropes release
   them, overlapped with the kT0/kT1/q00 ropes on DVE.
 - Attention segments (h,j) interleave the NEXT segment's q-projection
   matmuls between blocks so the PE never waits on ACT's exp stream
   (exp 612ns/block vs sc+av 427ns/block); the last segment instead
   pre-computes the first o-proj row block.
 - Weights are host-pre-arranged to contiguous per-partition layouts
   (no segmented-DMA penalty); wq is head-major and streamed per head;
   wo split per head so no transfer can block a rope half-swap DMA.
 - o-proj PSUM chunks rotate across all four pools (8 banks) so the
   ~1.1us PSUM->SBUF copy roundtrip never blocks the 852ns chains;
   the final row block's output DMA is split per 512 columns.
 - PSUM: 4 tags x 2 banks: sc, py, pq, pk (pk also hosts the [1,512]
   denominator rows and the late K(j2)/K(j3) chains; all slot
   rotations are creation-order A/B alternating and every tile's
   predecessor release is scheduled before its first use).
"""
import contextlib
from collections import deque

import numpy as np
import ml_dtypes

import concourse.bass as bass
import concourse.tile as tile
import concourse.mybir as mybir
from concourse.bass_utils import run_bass_kernel_spmd

BF16 = ml_dtypes.bfloat16

B, T, C = 2, 2048, 2048
H, HK, HD = 16, 4, 128
GQ = H // HK            # q heads per core = 4
NCORES = 8
TQC = 512               # tq chunk width
NTQ = T // TQC          # 4
NKC = C // 128          # 16 contraction chunks
NTK = T // 128          # 16 tk blocks
SCALE = 1.0 / float(np.sqrt(HD))

DT = mybir.dt.bfloat16
F32 = mybir.dt.float32


def _split_waits(nc, maxw=1):
    """This walrus build rejects instructions with >1 sync wait; move
    overflow waits onto same-engine nops inserted just before."""
    cnt = 0
    for f in nc.m.functions:
        for bb in f.blocks:
            idx = 0
            while idx < len(bb.instructions):
                inst = bb.instructions[idx]
                si = inst.sync_info
                waits = list(si.on_wait) if si is not None and si.on_wait else []
                if len(waits) > maxw:
                    updates = list(si.on_update) if si.on_update else []
                    keep, rest = waits[:maxw], waits[maxw:]
                    pos = idx
                    while rest:
                        chunk, rest = rest[:maxw], rest[maxw:]
                        cnt += 1
                        nop = mybir.InstNoOp(
                            name=f"waitsplit_{cnt}", engine=inst.engine,
                            ins=[], outs=[])
                        nop.sync_info = mybir.SyncInfo(on_wait=chunk, on_update=[])
                        nc.register_instruction(nop, overwrite=True)
                        bb.instructions.insert(pos, nop)
                        pos += 1
                        idx += 1
                    inst.sync_info = mybir.SyncInfo(on_wait=keep, on_update=updates)
                idx += 1
    return cnt


def build(reps: int = 1):
    nc = bass.Bass(target_bir_lowering=False)
    xTd = nc.dram_tensor("xT", [C, T], DT, kind="ExternalInput")
    cosT = nc.dram_tensor("cosT", [HD, T], DT, kind="ExternalInput")
    sinT = nc.dram_tensor("sinT", [HD, T], DT, kind="ExternalInput")
    wq = nc.dram_tensor("wq", [128, GQ * NKC * HD], DT, kind="ExternalInput")
    wk = nc.dram_tensor("wk", [128, NKC * HD], DT, kind="ExternalInput")
    wv = nc.dram_tensor("wv", [128, NKC * HD], DT, kind="ExternalInput")
    wo = nc.dram_tensor("wo", [128, GQ * C], DT, kind="ExternalInput")
    bqT = nc.dram_tensor("bqT", [HD, GQ], F32, kind="ExternalInput")
    bkT = nc.dram_tensor("bkT", [HD, 1], F32, kind="ExternalInput")
    bvr = nc.dram_tensor("bvr", [1, HD], F32, kind="ExternalInput")
    yp = nc.dram_tensor("yp", [T, C], DT, kind="ExternalOutput")
    rcscr = nc.dram_tensor("rcscr", [GQ * NTQ, TQC], F32)

    with tile.TileContext(nc) as tc, contextlib.ExitStack() as ctx:
        const = ctx.enter_context(tc.tile_pool(name="const", bufs=1))
        xtp = ctx.enter_context(tc.tile_pool(name="xtp", bufs=1))
        resid = ctx.enter_context(tc.tile_pool(name="resid", bufs=1))
        ytnp = ctx.enter_context(tc.tile_pool(name="ytnp", bufs=1))
        stage = ctx.enter_context(tc.tile_pool(name="stage", bufs=2))
        exs = ctx.enter_context(tc.tile_pool(name="exs", bufs=2))
        nrm = ctx.enter_context(tc.tile_pool(name="nrm", bufs=2))
        est = ctx.enter_context(tc.tile_pool(name="est", bufs=6))
        outp = ctx.enter_context(tc.tile_pool(name="outp", bufs=6))
        ps_sc = ctx.enter_context(tc.tile_pool(name="ps_sc", bufs=2, space="PSUM"))
        ps_py = ctx.enter_context(tc.tile_pool(name="ps_py", bufs=2, space="PSUM"))
        ps_pq = ctx.enter_context(tc.tile_pool(name="ps_pq", bufs=2, space="PSUM"))
        ps_pk = ctx.enter_context(tc.tile_pool(name="ps_pk", bufs=2, space="PSUM"))

        # ---- weights / constants (DMA queue order matters; see below) ----
        wk_all = const.tile([128, NKC * HD], DT)
        wv_all = const.tile([128, NKC * HD], DT)
        wq_all = const.tile([128, GQ * NKC * HD], DT)
        wo_all = const.tile([128, GQ * C], DT)
        wk_t = [wk_all[:, kc * HD:(kc + 1) * HD] for kc in range(NKC)]
        wv_t = [wv_all[:, kc * HD:(kc + 1) * HD] for kc in range(NKC)]
        wq_ht = [[wq_all[:, (h * NKC + kc) * HD:(h * NKC + kc + 1) * HD]
                  for kc in range(NKC)] for h in range(GQ)]
        wo_t = [wo_all[:, h * C:(h + 1) * C] for h in range(GQ)]
        bq_sb = const.tile([HD, GQ], F32)
        bk_sb = const.tile([HD, 1], F32)
        bvb_sb = const.tile([128, HD], F32)
        cos_sb = const.tile([HD, T], DT)
        sin_sb = const.tile([HD, T], DT)
        ones_sb = const.tile([128, 1], DT)
        nc.vector.memset(ones_sb, 1.0)

        # per-chunk resident tiles
        xt = [xtp.tile([128, T], DT, tag=f"xt{kc}", name=f"xt{kc}")
              for kc in range(NKC)]
        qTt = [[resid.tile([HD, TQC], DT, tag=f"qT{h}_{j}", name=f"qT{h}_{j}")
                for j in range(NTQ)] for h in range(GQ)]
        kTt = [resid.tile([HD, TQC], DT, tag=f"kT{j}", name=f"kT{j}")
               for j in range(NTQ)]
        vt = [resid.tile([128, HD], DT, tag=f"v{i}", name=f"v{i}")
              for i in range(NTK)]

        def rope_store(psum_src, bias_ap, dst_ap, j0, use_act=False):
            """dst = rope(psum_src + bias).

            sin_sb holds the half-swapped, sign-folded sin (host-prepped:
            rows 0:64 = sin[64:128], rows 64:128 = -sin[0:64]), so
            rot_half reduces to a full-width multiply followed by a
            partition half-swap done with two SBUF->SBUF DMAs.  The
            PSUM->SBUF bias-add runs on ACT (Identity shares the Exp
            table set, so no act-table reloads) unless ACT is the local
            bottleneck (j=3 segments), where it stays on DVE."""
            qs = stage.tile([128, TQC], DT, tag="qs")
            if use_act:
                nc.scalar.activation(
                    out=qs, in_=psum_src,
                    func=mybir.ActivationFunctionType.Identity, bias=bias_ap)
            else:
                nc.vector.tensor_scalar(
                    out=qs, in0=psum_src, scalar1=bias_ap, scalar2=None,
                    op0=mybir.AluOpType.add)
            tmp = stage.tile([128, TQC], DT, tag="tmp")
            nc.vector.tensor_mul(tmp, qs, cos_sb[:, j0:j0 + TQC])
            prod = stage.tile([128, TQC], DT, tag="prod")
            nc.vector.tensor_mul(prod, qs, sin_sb[:, j0:j0 + TQC])
            prodsw = stage.tile([128, TQC], DT, tag="prodsw")
            nc.sync.dma_start(out=prodsw[0:64, :], in_=prod[64:128, :])
            nc.sync.dma_start(out=prodsw[64:128, :], in_=prod[0:64, :])
            nc.vector.tensor_add(dst_ap, tmp, prodsw)

        for rep in range(reps):
            # ---- input DMAs.  Priority order: xt0+wk+wv first (first K
            # matmul), then the xt stream; small consts slot in early;
            # wq right after the stream (q-proj), wo last (o-proj).
            Q4 = 4 * HD
            if rep == 0:
                # first quarter of wk ahead of xt0 so the kc=0 K matmul
                # starts as soon as the first x chunk lands
                nc.scalar.dma_start(out=wk_all[:, 0:Q4], in_=wk[:, 0:Q4])
            nc.sync.dma_start(out=xt[0], in_=xTd[0:128, :])
            if rep == 0:
                nc.scalar.dma_start(out=wk_all[:, Q4:], in_=wk[:, Q4:])
                nc.scalar.dma_start(out=wv_all, in_=wv[:, :])
                nc.scalar.dma_start(out=wq_all[:, 0:NKC * HD],
                                    in_=wq[:, 0:NKC * HD])
            nc.sync.dma_start(out=xt[1], in_=xTd[128:256, :])
            if rep == 0:
                # j=0 slices of cos/sin (all the gate ropes need) + biases
                nc.scalar.dma_start(out=cos_sb[:, 0:TQC], in_=cosT[:, 0:TQC])
                nc.scalar.dma_start(out=sin_sb[:, 0:TQC], in_=sinT[:, 0:TQC])
                nc.scalar.dma_start(out=bq_sb, in_=bqT[:, :])
                nc.scalar.dma_start(out=bk_sb, in_=bkT[:, :])
                nc.scalar.dma_start(
                    out=bvb_sb, in_=bass.AP(bvr, 0, [[0, 128], [1, HD]]))
            for kc in range(2, NKC):
                eng = nc.sync if kc % 2 == 0 else nc.scalar
                eng.dma_start(out=xt[kc], in_=xTd[kc * 128:(kc + 1) * 128, :])
            if rep == 0:
                # after the xt stream (all on the ACT queue, keeping the SP
                # queue free for the rope half-swap DMAs): wq head 1 (s=0
                # fillers), cos/sin tails, remaining wq heads, wo last.
                nc.scalar.dma_start(
                    out=wq_all[:, NKC * HD:2 * NKC * HD],
                    in_=wq[:, NKC * HD:2 * NKC * HD])
                nc.scalar.dma_start(out=cos_sb[:, TQC:], in_=cosT[:, TQC:])
                nc.scalar.dma_start(out=sin_sb[:, TQC:], in_=sinT[:, TQC:])
                for hh in range(2, GQ):
                    nc.scalar.dma_start(
                        out=wq_all[:, hh * NKC * HD:(hh + 1) * NKC * HD],
                        in_=wq[:, hh * NKC * HD:(hh + 1) * NKC * HD])
                for hh in range(GQ):  # split: never block a rope swap long
                    nc.scalar.dma_start(out=wo_all[:, hh * C:(hh + 1) * C],
                                        in_=wo[:, hh * C:(hh + 1) * C])

            # ---- wave 1 (kc-major, rides the xt stream): K j=0,1 +
            # Q(h0,j0) + V blocks 0-3.  HARD CONSTRAINT: one PSUM
            # accumulation chain per bank at a time -- interleaving several
            # chains in one bank silently corrupts all but the last.
            pk01 = [ps_pk.tile([128, TQC], F32, tag="pk", name=f"pk{j}")
                    for j in range(2)]
            pq00 = ps_pq.tile([128, TQC], F32, tag="pq", name="pq00")
            pv = {0: ps_py.tile([128, TQC], F32, tag="py", name="pv0"),
                  1: ps_py.tile([128, TQC], F32, tag="py", name="pv1"),
                  2: ps_pq.tile([128, TQC], F32, tag="pq", name="pv2"),
                  3: ps_sc.tile([128, TQC], F32, tag="sc", name="pv3")}
            for kc in range(NKC):
                st, sp = kc == 0, kc == NKC - 1
                for j in range(2):
                    nc.tensor.matmul(pk01[j], wk_t[kc],
                                     xt[kc][:, j * TQC:(j + 1) * TQC],
                                     start=st, stop=sp)
                nc.tensor.matmul(pq00, wq_ht[0][kc],
                                 xt[kc][:, 0:TQC], start=st, stop=sp)
                for i in range(4):
                    nc.tensor.matmul(
                        pv[i][:, 0:HD],
                        xt[kc][:, i * 128:(i + 1) * 128], wv_t[kc],
                        start=st, stop=sp)

            # gate: V4-15 as 12 solo kc-inner chains rotating through banks
            # as the bias-adds / ropes release them, overlapping the kT0 +
            # q00 ropes on DVE.  PE order below matches each chain's slot
            # release (see the DVE emission order that follows).
            def v_chain(i, pool, tag):
                t = pool.tile([128, TQC], F32, tag=tag, name=f"pv{i}")
                pv[i] = t
                for kc in range(NKC):
                    nc.tensor.matmul(
                        t[:, 0:HD], xt[kc][:, i * 128:(i + 1) * 128],
                        wv_t[kc], start=(kc == 0), stop=(kc == NKC - 1))

            def vt_add(i):
                nc.vector.tensor_add(vt[i], pv[i][:, 0:HD], bvb_sb)

            # PE: chains in slot-release order
            v_chain(4, ps_sc, "sc")      # sc-B, free
            for i in range(4):           # DVE: vt0-3 first (release w1 slots)
                vt_add(i)
            rope_store(pk01[0], bk_sb[:, 0:1], kTt[0], 0)
            v_chain(5, ps_sc, "sc")      # sc-A <- vt3
            v_chain(6, ps_py, "py")      # py-A <- vt0
            v_chain(7, ps_py, "py")      # py-B <- vt1
            vt_add(4)
            vt_add(5)
            vt_add(6)
            vt_add(7)
            rope_store(pq00, bq_sb[:, 0:1], qTt[0][0], 0)
            v_chain(14, ps_pq, "pq")     # pq-B <- vt? (v2 add above)  [B]
            v_chain(10, ps_sc, "sc")     # sc-B <- vt4
            v_chain(11, ps_sc, "sc")     # sc-A <- vt5
            v_chain(12, ps_py, "py")     # py-A <- vt6
            v_chain(13, ps_py, "py")     # py-B <- vt7
            vt_add(14)
            vt_add(10)
            vt_add(11)
            vt_add(12)
            vt_add(13)
            rope_store(pk01[1], bk_sb[:, 0:1], kTt[1], TQC)
            v_chain(8, ps_pq, "pq")      # pq-A <- rope q00
            v_chain(9, ps_pk, "pk")      # pk-B <- rope kT1 (after sums00=A)
            vt_add(8)
            vt_add(9)
            v_chain(15, ps_pq, "pq")     # pq-B <- vt14
            vt_add(15)

            # ---- attention segments ----
            # seg order: j outer, h inner.  seg s handles (h,j); during its
            # blocks the PE is fed fillers: next segment's q-proj, plus the
            # deferred V12-15 / K j2 / K j3 chains.
            segs = [(h, j) for j in range(NTQ) for h in range(GQ)]
            sums_j = {}
            pq_next = {0: pq00}
            # deferred PE chains, emitted as after-block fillers:
            #   seg 0 -> V12-15 (py rotation); seg 5 -> K j2 (pk rotation);
            #   seg 9 -> K j3 (pk rotation)
            # DVE extras after each segment's next-q rope:
            #   seg 0 -> vt8-11; seg 1 -> kT1, vt12-15; seg 2 -> vt4-7;
            #   seg 6 -> kT2; seg 10 -> kT3
            pk_late = {}
            ot00 = {}

            ytn_all = {j: [None] * GQ for j in range(NTQ)}
            for s, (h, j) in enumerate(segs):
                j0 = j * TQC
                nblk = 4 * j + 4
                # fillers: next segment's q-projection matmul thunks; the
                # last segment instead pre-computes the first o-proj row
                # block (its q-heads' yt are long since done)
                fill = deque()
                if s + 1 < len(segs):
                    nh, nj = segs[s + 1]
                    pqn = ps_pq.tile([128, TQC], F32, tag="pq",
                                     name=f"pq{nh}{nj}")
                    pq_next[s + 1] = pqn
                    for kc in range(NKC):
                        fill.append((lambda kc=kc, pqn=pqn, nh=nh, nj=nj:
                                     nc.tensor.matmul(
                                         pqn, wq_ht[nh][kc],
                                         xt[kc][:, nj * TQC:(nj + 1) * TQC],
                                         start=(kc == 0), stop=(kc == NKC - 1))))
                else:
                    ot00[0] = outp.tile([128, C], DT, name="ot00", bufs=1)
                    for cc in range(4):
                        po = ps_pq.tile([128, TQC], F32, tag="pq",
                                        name="po_pre")
                        for hh in range(GQ):
                            def mk(po=po, hh=hh, c0=cc * TQC):
                                nc.tensor.matmul(
                                    po, ytn_all[0][hh][:, 0:128],
                                    wo_t[hh][:, c0:c0 + TQC],
                                    start=(hh == 0), stop=(hh == GQ - 1))
                                if hh == GQ - 1:
                                    nc.vector.tensor_copy(
                                        out=ot00[0][:, c0:c0 + TQC], in_=po)
                            fill.append(mk)

                def after_fill():
                    while fill:
                        fill.popleft()()
                    if s in (5, 9):
                        jj = 2 if s == 5 else 3
                        pkl = ps_pk.tile([128, TQC], F32, tag="pk",
                                         name=f"pkl{jj}")
                        pk_late[jj] = pkl
                        for kc in range(NKC):
                            nc.tensor.matmul(
                                pkl, wk_t[kc],
                                xt[kc][:, jj * TQC:(jj + 1) * TQC],
                                start=(kc == 0), stop=(kc == NKC - 1))

                sums_hj = ps_pk.tile([1, TQC], F32, tag="pk",
                                     name=f"sums{h}{j}")
                sums_j[(h, j)] = sums_hj
                py = ps_py.tile([HD, TQC], F32, tag="py", name=f"py{h}{j}")

                # blocks with a 2-deep sc pipeline; group-of-4 tree sums
                ngrp = (nblk + 3) // 4
                ex_t = [None] * nblk
                sc_t = [None] * nblk
                pend_sum = deque()   # (grp, exsum_tile) ready for ones-matmul

                def emit_sc(i):
                    srel = i - 4 * j
                    c0 = 128 * srel if srel > 0 else 0
                    sct = ps_sc.tile([128, TQC], F32, tag="sc")
                    sc_t[i] = (sct, c0)
                    jk, ik = divmod(i, 4)
                    nc.tensor.matmul(
                        sct[:, c0:TQC], kTt[jk][:, ik * 128:(ik + 1) * 128],
                        qTt[h][j][:, c0:TQC], start=True, stop=True)
                    ex = est.tile([128, TQC], DT)
                    ex_t[i] = ex
                    nc.scalar.activation(
                        out=ex[:, c0:TQC], in_=sct[:, c0:TQC],
                        func=mybir.ActivationFunctionType.Exp, scale=SCALE)
                    if srel >= 0:
                        if srel > 0:
                            nc.gpsimd.memset(ex[:, 0:c0], 0.0)
                        # zero below-diagonal of the 128-wide diag sub-block
                        nc.gpsimd.affine_select(
                            out=ex[:, c0:c0 + 128], in_=ex[:, c0:c0 + 128],
                            compare_op=mybir.AluOpType.is_ge, fill=0.0,
                            base=0, pattern=[[1, 128]], channel_multiplier=-1)

                def emit_group_sum(g):
                    lo = g * 4
                    hi = min(lo + 4, nblk)
                    tiles = [ex_t[i] for i in range(lo, hi)]
                    if len(tiles) == 4:
                        t01 = exs.tile([128, TQC], DT, tag="t01")
                        t23 = exs.tile([128, TQC], DT, tag="t23")
                        nc.vector.tensor_add(t01, tiles[0], tiles[1])
                        nc.vector.tensor_add(t23, tiles[2], tiles[3])
                        nc.vector.tensor_add(t01, t01, t23)
                        acc = t01
                    elif len(tiles) == 2:
                        t01 = exs.tile([128, TQC], DT, tag="t01")
                        nc.vector.tensor_add(t01, tiles[0], tiles[1])
                        acc = t01
                    else:
                        acc = tiles[0]  # unreachable for nblk % 4 == 0
                    pend_sum.append((g, acc))

                def emit_pend_sums():
                    while pend_sum:
                        g, acc = pend_sum.popleft()
                        nc.tensor.matmul(
                            sums_hj, ones_sb, acc,
                            start=(g == 0), stop=(g == ngrp - 1))

                emit_sc(0)
                if nblk > 1:
                    emit_sc(1)
                for i in range(nblk):
                    if fill:
                        fill.popleft()()
                    sct, c0 = sc_t[i]
                    nc.tensor.matmul(
                        py[:, c0:TQC], vt[i], ex_t[i][:, c0:TQC],
                        start=(i == 0), stop=(i == nblk - 1))
                    if i + 2 < nblk:
                        emit_sc(i + 2)
                    if i % 4 == 3:
                        emit_group_sum(i // 4)
                    if i % 4 == 1 and i > 4:
                        emit_pend_sums()
                after_fill()
                emit_pend_sums()

                # DVE tail: next-q rope first (gates next segment), then
                # scheduled extras, then normalization of this segment.
                if s + 1 < len(segs):
                    nh, nj = segs[s + 1]
                    # j=3 segments are ACT-limited (16 exps); keep their
                    # rope's PSUM->SBUF step on DVE instead
                    rope_store(pq_next[s + 1], bq_sb[:, nh:nh + 1],
                               qTt[nh][nj], nj * TQC, use_act=(nj != 3))
                if s == 6:
                    rope_store(pk_late[2], bk_sb[:, 0:1], kTt[2], 2 * TQC)
                elif s == 10:
                    rope_store(pk_late[3], bk_sb[:, 0:1], kTt[3], 3 * TQC)

                # normalize: yT[d, tq] / sum[tq] via DRAM-broadcast of 1/sum
                rc = nrm.tile([1, TQC], F32, tag="rc")
                nc.vector.reciprocal(out=rc, in_=sums_hj[0:1, :])
                rcb = nrm.tile([HD, TQC], F32, tag="rcb")
                idx = h * NTQ + j
                nc.scalar.dma_start(out=rcscr[idx:idx + 1, :], in_=rc)
                nc.scalar.dma_start(
                    out=rcb, in_=bass.AP(rcscr, idx * TQC, [[0, HD], [1, TQC]]))
                yt = ytnp.tile([HD, TQC], DT, tag=f"yt{h}_{j}",
                               name=f"yt{h}_{j}")
                nc.vector.tensor_mul(yt, py, rcb)
                ytn_all[j][h] = yt

            # ---- o-proj: y[tq, :] = sum_h yT_h.T @ Wo_h.  po chunks rotate
            # across all 4 PSUM pools (8 banks) so the ~1.1us PSUM->SBUF
            # copy roundtrip never blocks the 852ns accumulation chains.
            po_pools = [(ps_pq, "pq"), (ps_sc, "sc"), (ps_py, "py"),
                        (ps_pk, "pk")]
            po_i = 0
            nc.sync.dma_start(out=yp[0:128, :], in_=ot00[0])
            for j in range(NTQ):
                ytn = ytn_all[j]
                for t in range(4):  # four 128-row q tiles in this chunk
                    if j == 0 and t == 0:
                        continue  # pre-computed during the last segment
                    trow = j * TQC + t * 128
                    last = (j == NTQ - 1 and t == 3)
                    ot = outp.tile([128, C], DT)
                    for cc in range(4):
                        c0 = cc * TQC
                        pool, ptag = po_pools[po_i % 4]
                        po_i += 1
                        po = pool.tile([128, TQC], F32, tag=ptag, name="po")
                        for hh in range(GQ):
                            nc.tensor.matmul(
                                po, ytn[hh][:, t * 128:(t + 1) * 128],
                                wo_t[hh][:, c0:c0 + TQC],
                                start=(hh == 0), stop=(hh == GQ - 1))
                        if (t + cc) % 2 == 0:
                            nc.scalar.copy(out=ot[:, c0:c0 + TQC], in_=po)
                        else:
                            nc.vector.tensor_copy(out=ot[:, c0:c0 + TQC], in_=po)
                        if last:
                            # fine-grained tail: expose only a 512-col DMA
                            oeng = nc.sync if cc % 2 == 0 else nc.scalar
                            oeng.dma_start(
                                out=yp[trow:trow + 128, c0:c0 + TQC],
                                in_=ot[:, c0:c0 + TQC])
                    if not last:
                        oeng = nc.sync if t % 2 == 0 else nc.scalar
                        oeng.dma_start(out=yp[trow:trow + 128, :], in_=ot)
    _split_waits(nc, maxw=1)
    return nc


def _in_maps(x, cos, sin, Wq, bq, Wk, bk, Wv, bv, Wo):
    maps = []
    for c in range(NCORES):
        b, g = divmod(c, HK)
        qsl = slice(g * GQ * HD, (g + 1) * GQ * HD)
        ksl = slice(g * HD, (g + 1) * HD)
        maps.append({
            "xT": np.ascontiguousarray(x[b].T.astype(BF16)),
            "cosT": np.ascontiguousarray(cos[b].T.astype(BF16)),
            "sinT": np.ascontiguousarray(np.concatenate(
                [sin[b].T[64:128], -sin[b].T[0:64]], axis=0).astype(BF16)),
            # weights pre-arranged to the exact SBUF layout so every DMA is
            # a contiguous per-partition transfer:
            #   wq: [128p, h, kc, HD] head-major; wk/wv: [128p, kc, HD];
            #   wo: [128p(=HD), h, C]
            "wq": np.ascontiguousarray(
                Wq[:, qsl].reshape(NKC, 128, GQ, HD).transpose(1, 2, 0, 3)
                .reshape(128, GQ * NKC * HD).astype(BF16)),
            "wk": np.ascontiguousarray(
                Wk[:, ksl].reshape(NKC, 128, HD).transpose(1, 0, 2)
                .reshape(128, NKC * HD).astype(BF16)),
            "wv": np.ascontiguousarray(
                Wv[:, ksl].reshape(NKC, 128, HD).transpose(1, 0, 2)
                .reshape(128, NKC * HD).astype(BF16)),
            "wo": np.ascontiguousarray(
                Wo[qsl, :].reshape(GQ, 128, C).transpose(1, 0, 2)
                .reshape(128, GQ * C).astype(BF16)),
            "bqT": np.ascontiguousarray(
                bq[qsl].reshape(GQ, HD).T.astype(np.float32)),
            "bkT": np.ascontiguousarray(
                bk[ksl].reshape(HD, 1).astype(np.float32)),
            "bvr": np.ascontiguousarray(
                bv[ksl].reshape(1, HD).astype(np.float32)),
        })
    return maps


_nc_cache = {}


def kernel(x, cos, sin, Wq, bq, Wk, bk, Wv, bv, Wo):
    x, cos, sin = np.asarray(x), np.asarray(cos), np.asarray(sin)
    Wq, bq = np.asarray(Wq), np.asarray(bq)
    Wk, bk = np.asarray(Wk), np.asarray(bk)
    Wv, bv = np.asarray(Wv), np.asarray(bv)
    Wo = np.asarray(Wo)
    if "nc" not in _nc_cache:
        _nc_cache["nc"] = build(reps=1)
    nc = _nc_cache["nc"]
    maps = _in_maps(x, cos, sin, Wq, bq, Wk, bk, Wv, bv, Wo)
    res = run_bass_kernel_spmd(nc, maps, core_ids=list(range(NCORES)))
    out = np.zeros((B, T, C), dtype=np.float32)
    for c in range(NCORES):
        b = c // HK
        out[b] += res.results[c]["yp"].astype(np.float32)
    return out
